# revision 15
# baseline (speedup 1.0000x reference)
"""DeepseekV2 MoE kernel for 8 trn2 NeuronCores (expert-parallel).

Strategy:
  - Router (gate matmul + softmax + group-limited top-k) runs on host in
    jax-on-CPU, replicating the module's math op-for-op.
  - Tokens are gathered per expert (capacity 256 per expert; actual
    per-expert load for T=1024/K=6/E=32 is ~192) and dispatched
    expert-parallel: core c owns experts 4c..4c+3.
  - Each core runs silu(x@w1.T)*(x@w3.T)@w2.T for its 4 experts over
    their gathered tokens, everything laid out with tokens on the matmul
    free dim so no on-device transposes are needed.
  - Host scatter-adds the per-(token,expert) outputs with the routing
    weights. Capacity overflow (not expected for the target shapes) is
    computed on host as a correctness fallback.
"""

import os
import numpy as np

import concourse.bass as bass
import concourse.mybir as mybir
import concourse.tile as tile
from concourse import bacc

E, G, TG, TOPK = 32, 8, 3, 6
H, I, T = 2048, 1408, 1024
N_CORES = 8
EPC = E // N_CORES          # experts per core
CAP = 256                   # token capacity per expert
KT, IT, HT = H // 128, I // 128, H // 128   # 16, 11, 16 k/i/h tiles

# matmul dtype mode: "f32", "f32r", "bf16", or "f16"
MM_MODE = os.environ.get("MOE_MM_MODE", "f32r")

_prog_cache = {}


def _build_program(mode, repeat=1, loop_reps=0):
    """Per-core SPMD program: 4 experts x (CAP tokens) gated FFN.

    repeat>1 re-runs the whole computation unrolled (identical outputs);
    loop_reps>0 wraps it in a hardware For_i loop instead. Both exist so
    wall-time deltas isolate device time from dispatch overhead when
    profiling."""
    f32 = mybir.dt.float32
    store_dt = {"bf16": mybir.dt.bfloat16,
                "f16": mybir.dt.float16,
                "f32r": mybir.dt.float32r}.get(mode, f32)

    def mm(ap):
        return ap

    nc = bacc.Bacc("TRN2", target_bir_lowering=False, debug=False,
                   num_devices=N_CORES)

    # Blocked layouts (see host prep below):
    #   xb  [KT, 128, EPC*CAP]   xb[k,p,t] = x_gathered[t, 128k+p]
    #   w1b [EPC, IT, 128, KT*128] strip rows p = h within k-tile
    #   w3b same as w1b
    #   w2b [EPC, HT, 128, IT*128] strip rows p = i within i-tile
    #   yb  [HT, 128, EPC*CAP]   yb[h,p,t] = y[t, 128h+p]
    xb = nc.dram_tensor("xb", [KT, 128, EPC * CAP], store_dt,
                        kind="ExternalInput").ap()
    w1b = nc.dram_tensor("w1b", [EPC, IT, 128, KT * 128], store_dt,
                         kind="ExternalInput").ap()
    w3b = nc.dram_tensor("w3b", [EPC, IT, 128, KT * 128], store_dt,
                         kind="ExternalInput").ap()
    w2b = nc.dram_tensor("w2b", [EPC, HT, 128, IT * 128], store_dt,
                         kind="ExternalInput").ap()
    yb = nc.dram_tensor("yb", [HT, 128, EPC * CAP], f32,
                        kind="ExternalOutput").ap()

    with tile.TileContext(nc) as tc:
        with (
            tc.tile_pool(name="xpool", bufs=1) as xpool,
            tc.tile_pool(name="wpool", bufs=3) as wpool,
            tc.tile_pool(name="hhpool", bufs=2 * IT + 1) as hhpool,
            tc.tile_pool(name="evpool", bufs=4) as evpool,
            tc.tile_pool(name="psum", bufs=6, space="PSUM") as psum,
        ):
            # Resident gathered activations: [128, KT*EPC*CAP]
            x_sb = xpool.tile([128, KT * EPC * CAP], store_dt, tag="x")
            for k in range(KT):
                nc.sync.dma_start(
                    x_sb[:, k * EPC * CAP:(k + 1) * EPC * CAP], xb[k])

            def body():
              for e in range(EPC):
                tok = bass.ds(e * CAP, CAP)
                hh = []
                for it in range(IT):
                    w1s = wpool.tile([128, KT * 128], store_dt, tag="w1s")
                    nc.sync.dma_start(w1s[:], w1b[e, it])
                    w3s = wpool.tile([128, KT * 128], store_dt, tag="w3s")
                    nc.sync.dma_start(w3s[:], w3b[e, it])

                    psA = psum.tile([128, CAP], f32, tag="ps")
                    for k in range(KT):
                        nc.tensor.matmul(
                            psA[:],
                            mm(w1s[:, bass.ts(k, 128)]),
                            mm(x_sb[:, bass.ds(k * EPC * CAP + e * CAP, CAP)]),
                            start=(k == 0), stop=(k == KT - 1))
                    psB = psum.tile([128, CAP], f32, tag="ps")
                    for k in range(KT):
                        nc.tensor.matmul(
                            psB[:],
                            mm(w3s[:, bass.ts(k, 128)]),
                            mm(x_sb[:, bass.ds(k * EPC * CAP + e * CAP, CAP)]),
                            start=(k == 0), stop=(k == KT - 1))

                    sA = evpool.tile([128, CAP], f32, tag="silu")
                    nc.scalar.activation(
                        sA[:], psA[:], mybir.ActivationFunctionType.Silu)
                    hh_t = hhpool.tile([128, CAP], store_dt, tag="hh")
                    nc.vector.tensor_mul(hh_t[:], sA[:], psB[:])
                    hh.append(hh_t)

                for ht in range(HT):
                    w2s = wpool.tile([128, IT * 128], store_dt, tag="w2s")
                    nc.sync.dma_start(w2s[:], w2b[e, ht])
                    psY = psum.tile([128, CAP], f32, tag="ps")
                    for it2 in range(IT):
                        nc.tensor.matmul(
                            psY[:],
                            mm(w2s[:, bass.ts(it2, 128)]),
                            mm(hh[it2][:]),
                            start=(it2 == 0), stop=(it2 == IT - 1))
                    yo = evpool.tile([128, CAP], f32, tag="yo")
                    nc.vector.tensor_copy(yo[:], psY[:])
                    nc.sync.dma_start(yb[ht][:, tok], yo[:])

            if loop_reps > 0:
                with tc.For_i(0, loop_reps, 1,
                              hint_engines=(mybir.EngineType.PE,
                                            mybir.EngineType.SP)):
                    body()
            else:
                for _ in range(repeat):
                    body()
    nc.compile()
    return nc


def get_program(mode=None, repeat=1, loop_reps=0):
    mode = mode or MM_MODE
    key = (mode, repeat, loop_reps)
    if key not in _prog_cache:
        _prog_cache[key] = _build_program(mode, repeat, loop_reps)
    return _prog_cache[key]


_exec_cache = {}


def get_executor(mode=None, repeat=1, loop_reps=0):
    """Build (once) a PJRT executable for the SPMD program. Returns a
    callable: in_maps (list of per-core dicts) -> list of per-core output
    dicts."""
    mode = mode or MM_MODE
    key = (mode, repeat, loop_reps)
    if key in _exec_cache:
        return _exec_cache[key]

    import jax
    from jax.sharding import Mesh, NamedSharding, PartitionSpec
    from jax.experimental.shard_map import shard_map
    from concourse import bass2jax

    bass2jax.install_neuronx_cc_hook()
    nc = get_program(mode, repeat, loop_reps)

    partition_name = (nc.partition_id_tensor.name
                      if nc.partition_id_tensor else None)
    in_names, out_names, out_avals, out_shapes = [], [], [], []
    for alloc in nc.m.functions[0].allocations:
        if not isinstance(alloc, mybir.MemoryLocationSet):
            continue
        name = alloc.memorylocations[0].name
        if alloc.kind == "ExternalInput":
            if name != partition_name:
                in_names.append(name)
        elif alloc.kind == "ExternalOutput":
            shape = tuple(alloc.tensor_shape)
            dtype = mybir.dt.np(alloc.dtype)
            out_names.append(name)
            out_avals.append(jax.core.ShapedArray(shape, dtype))
            out_shapes.append((shape, dtype))
    n_params = len(in_names)
    n_outs = len(out_avals)
    all_in_names = in_names + out_names + (
        [partition_name] if partition_name else [])

    def _body(*args):
        operands = list(args)
        if partition_name is not None:
            operands.append(bass2jax.partition_id_tensor())
        return tuple(bass2jax._bass_exec_p.bind(
            *operands,
            out_avals=tuple(out_avals),
            in_names=tuple(all_in_names),
            out_names=tuple(out_names),
            lowering_input_output_aliases=(),
            sim_require_finite=True,
            sim_require_nnan=True,
            nc=nc,
        ))

    devices = jax.devices()[:N_CORES]
    mesh = Mesh(np.asarray(devices), ("core",))
    sharded = jax.jit(
        shard_map(_body, mesh=mesh,
                  in_specs=(PartitionSpec("core"),) * (n_params + n_outs),
                  out_specs=(PartitionSpec("core"),) * n_outs,
                  check_rep=False),
        donate_argnums=tuple(range(n_params, n_params + n_outs)),
        keep_unused=True)
    shard = NamedSharding(mesh, PartitionSpec("core"))

    def run(in_maps):
        concat_in = [
            np.concatenate([np.asarray(in_maps[c][nm])
                            for c in range(N_CORES)], axis=0)
            for nm in in_names]
        zeros = [np.zeros((N_CORES * s[0], *s[1:]), d)
                 for (s, d) in out_shapes]
        outs = sharded(*[jax.device_put(a, shard) for a in concat_in],
                       *[jax.device_put(z, shard) for z in zeros])
        return [
            {name: np.asarray(outs[i]).reshape(N_CORES, *out_avals[i].shape)[c]
             for i, name in enumerate(out_names)}
            for c in range(N_CORES)]

    run.in_names = in_names
    run.out_names = out_names
    run.out_shapes = out_shapes
    run.sharded = sharded
    run.shard = shard
    _exec_cache[key] = run
    return run


def _route(hidden_states, gate_weight):
    """Replicates the module's router on CPU via jax (bit-compatible with
    the reference implementation)."""
    import jax
    import jax.numpy as jnp
    cpu = jax.devices("cpu")[0]
    with jax.default_device(cpu):
        hs = jnp.asarray(hidden_states)
        gw = jnp.asarray(gate_weight)
        logits = hs @ gw.T
        probs = jax.nn.softmax(logits.astype(jnp.float32), axis=-1)
        group_scores = probs.reshape(T, G, E // G).max(axis=-1)
        _, gidx = jax.lax.top_k(group_scores, TG)
        rows = jnp.arange(T)[:, None]
        gmask = jnp.zeros((T, G), probs.dtype).at[rows, gidx].set(1.0)
        smask = jnp.repeat(gmask, E // G, axis=1)
        tmp_scores = jnp.where(smask > 0, probs, 0.0)
        rw, sel = jax.lax.top_k(tmp_scores, TOPK)
        return np.asarray(sel), np.asarray(rw, dtype=np.float32)


def _np_store_dtype(mode):
    if mode == "bf16":
        import ml_dtypes
        return np.dtype(ml_dtypes.bfloat16)
    if mode == "f16":
        return np.dtype(np.float16)
    return np.dtype(np.float32)


def prep_inputs(hidden_states, w1_weight, w3_weight, w2_weight, sel, mode):
    """Gather tokens per expert + block weights for the device layout.
    Returns (in_maps, assign, overflow) where assign[t,k] = slot row in the
    global gathered array or -1 if overflowed."""
    sdt = _np_store_dtype(mode)
    assign = np.full((T, TOPK), -1, dtype=np.int64)
    counts = np.zeros(E, dtype=np.int64)
    overflow = []
    tok_of = np.full((E, CAP), 0, dtype=np.int64)
    used = np.zeros((E, CAP), dtype=bool)
    for t in range(T):
        for k in range(TOPK):
            e = sel[t, k]
            c = counts[e]
            if c < CAP:
                tok_of[e, c] = t
                used[e, c] = True
                counts[e] = c + 1
                assign[t, k] = e * CAP + c
            else:
                overflow.append((t, k, e))

    in_maps = []
    for core in range(N_CORES):
        es = slice(core * EPC, (core + 1) * EPC)
        # gathered x: [EPC*CAP, H]
        xg = np.zeros((EPC * CAP, H), dtype=np.float32)
        idx = tok_of[es].reshape(-1)
        msk = used[es].reshape(-1)
        xg[msk] = hidden_states[idx[msk]]
        xbc = np.ascontiguousarray(
            xg.T.reshape(KT, 128, EPC * CAP)).astype(sdt, copy=False)
        # weights: w1/w3 [e, I, H] -> strips [e, it, p(h within kt), kt*128+i]
        w1c = np.ascontiguousarray(
            w1_weight[es].transpose(0, 2, 1)          # [e, H, I]
            .reshape(EPC, KT, 128, IT, 128)           # [e, kt, p, it, i]
            .transpose(0, 3, 2, 1, 4)                 # [e, it, p, kt, i]
            .reshape(EPC, IT, 128, KT * 128)).astype(sdt, copy=False)
        w3c = np.ascontiguousarray(
            w3_weight[es].transpose(0, 2, 1)
            .reshape(EPC, KT, 128, IT, 128)
            .transpose(0, 3, 2, 1, 4)
            .reshape(EPC, IT, 128, KT * 128)).astype(sdt, copy=False)
        # w2 [e, H, I] -> strips [e, ht, p(i within it), it*128+h]
        w2c = np.ascontiguousarray(
            w2_weight[es].transpose(0, 2, 1)          # [e, I, H]
            .reshape(EPC, IT, 128, HT, 128)           # [e, it, p, ht, h]
            .transpose(0, 3, 2, 1, 4)                 # [e, ht, p, it, h]
            .reshape(EPC, HT, 128, IT * 128)).astype(sdt, copy=False)
        in_maps.append({"xb": xbc, "w1b": w1c, "w3b": w3c, "w2b": w2c})
    return in_maps, assign, overflow


def combine(results, assign, rw, overflow, hidden_states,
            w1_weight, w3_weight, w2_weight):
    # Global gathered output rows: core-major [N_CORES*EPC*CAP, H]
    ys = []
    for core in range(N_CORES):
        ycore = results[core]["yb"].reshape(H, EPC * CAP).T  # [EPC*CAP, H]
        ys.append(ycore)
    yg = np.concatenate(ys, axis=0)                          # [E*CAP, H]

    flat = assign.reshape(-1)
    ok = flat >= 0
    picked = np.zeros((T * TOPK, H), dtype=np.float32)
    picked[ok] = yg[flat[ok]]
    out = (picked.reshape(T, TOPK, H)
           * rw[:, :, None]).sum(axis=1).astype(np.float32)

    if overflow:
        for (t, k, e) in overflow:
            x = hidden_states[t]
            h = (x @ w1_weight[e].T)
            h = (h / (1.0 + np.exp(-h))) * (x @ w3_weight[e].T)
            out[t] += rw[t, k] * (h @ w2_weight[e].T)
    return out


def kernel(hidden_states, gate_weight, w1_weight, w3_weight, w2_weight):
    mode = MM_MODE
    runner = get_executor(mode)
    sel, rw = _route(hidden_states, gate_weight)
    in_maps, assign, overflow = prep_inputs(
        hidden_states, w1_weight, w3_weight, w2_weight, sel, mode)
    results = runner(in_maps)
    return combine(results, assign, rw, overflow, hidden_states,
                   w1_weight, w3_weight, w2_weight)
